# revision 17
# baseline (speedup 1.0000x reference)
"""Trainium2 Bass kernel for nn_CrossAttentionClassifier.

Strategy
--------
The reference network with q_len = kv_len = 1 attention degenerates into a
pure MLP:

    z_m = mut @ Wm' + bm'          (centered: LN mean-subtraction folded into W)
    z_c = ctx @ Wc' + bc'
    h_m = relu(z_m * rsqrt(mean(z_m^2)+eps)) ; h_c likewise
    pre1 = h_c @ CA + h_m @ CM + d (attention V/out projections + c1 folded)
    z1   = pre1 * rsqrt(mean(pre1^2)+eps)
    h1   = relu(z1 * g + be)
    h2   = relu(h1 @ c2_w + c2_b)
    out  = h2 @ c3_w + c3_b

All weight folding (products of the tiny 256x256 projection chains and the
centering projector I - 11^T/256) happens on host in float64; the batch-heavy
work runs on 8 NeuronCores, data-parallel over the 65536-row batch.

Device layout: batch on SBUF partitions, features on the free axis, so both
layernorms reduce along the free dimension (bn_stats).  Activations are
transposed 128x128 via the PE between layers so the next matmul's contraction
dim lands on partitions.  Biases of the big embed matmuls ride along as an
appended constant-one feature row.  All matmuls run in bf16 (fp32 PSUM
accumulate).

The per-block work is software-pipelined three deep (A: embed matmuls,
B: z transposes + mid matmul, C: z1 transposes + c2/c3) so the Tensor
engine's in-order stream never waits on the DVE/ACT layernorm chains —
keeping PE HAM-warm.  All constants arrive in two packed DMAs (walrus
allows only one sync-wait per instruction; see _legalize_waits).
"""

import numpy as np
import ml_dtypes

import concourse.bass as bass
import concourse.mybir as mybir
import concourse.tile as tile
from concourse.bass_utils import run_bass_kernel_spmd

BF16 = ml_dtypes.bfloat16
F32 = np.float32

N_CORES = 8
B = 65536
IN_DIM = 2056
E = 256
EPS = 1e-5
KP = 2176          # feature dim padded to 17*128 (incl. bias row at 2056)
KCH = KP // 128    # 17
ROWS = B // N_CORES   # 8192 rows per core
ST = 512           # batch columns per supertile
NST = ROWS // ST   # 16
NBB = ST // 128    # 4 blocks of 128 per supertile

_BF = mybir.dt.bfloat16
_F32 = mybir.dt.float32

# ---- packed bf16 const image column offsets ----
_WM0 = 0
_WC0 = _WM0 + KCH * E          # 4352
_WMID0 = _WC0 + KCH * E        # 8704
_C2W0 = _WMID0 + 4 * E         # 9728
_C3W0 = _C2W0 + 2 * 64         # 9856
_D0 = _C3W0 + 1                # 9857
_ONES0 = _D0 + E               # 10113
_ID0 = _ONES0 + 128            # 10241
_WCOLS = _ID0 + 128            # 10369

# ---- packed f32 const image columns ----
# 0: g chunk0, 1: g chunk1, 2: be chunk0, 3: be chunk1, 4: eps,
# 5: c2_b (64 valid), 6: c3_b (1 valid), 7: pad, 8:8+E: d broadcast
_FCOLS = 8 + E


def _build_nc():
    nc = bass.Bass()

    xm = nc.dram_tensor("xm", [KP, ROWS], _BF, kind="ExternalInput")
    xc = nc.dram_tensor("xc", [KP, ROWS], _BF, kind="ExternalInput")
    wpack = nc.dram_tensor("wpack", [128, _WCOLS], _BF, kind="ExternalInput")
    fpack = nc.dram_tensor("fpack", [128, _FCOLS], _F32, kind="ExternalInput")
    out = nc.dram_tensor("out", [1, ROWS], _F32, kind="ExternalOutput")

    from contextlib import ExitStack

    with tile.TileContext(nc) as tc, ExitStack() as ctx:
        consts = ctx.enter_context(tc.tile_pool(name="consts", bufs=1))
        xpool = ctx.enter_context(tc.tile_pool(name="xpool", bufs=2))
        zpool = ctx.enter_context(tc.tile_pool(name="zpool", bufs=3))
        hpool = ctx.enter_context(tc.tile_pool(name="hpool", bufs=4))
        spool = ctx.enter_context(tc.tile_pool(name="spool", bufs=6))
        opool = ctx.enter_context(tc.tile_pool(name="opool", bufs=3))
        pe_mc = ctx.enter_context(tc.tile_pool(name="pe_mc", bufs=2, space="PSUM"))
        pe_p1 = ctx.enter_context(tc.tile_pool(name="pe_p1", bufs=2, space="PSUM"))
        pe_t = ctx.enter_context(tc.tile_pool(name="pe_t", bufs=2, space="PSUM"))
        pe_small = ctx.enter_context(tc.tile_pool(name="pe_small", bufs=2, space="PSUM"))

        wsb = consts.tile([128, _WCOLS], _BF, tag="wsb")
        fsb = consts.tile([128, _FCOLS], _F32, tag="fsb")

        def load_consts():
            # wm/wc in quarters so the first embed matmuls start early;
            # emitted interleaved with supertile 0's per-block x loads.
            qs = [(0, 5), (5, 9), (9, 13), (13, KCH)]
            for base in (_WM0, _WC0):
                for k0, k1 in qs:
                    nc.sync.dma_start(
                        out=wsb[:, base + k0 * E:base + k1 * E],
                        in_=wpack[:, base + k0 * E:base + k1 * E])
            nc.sync.dma_start(out=fsb, in_=fpack[:])
            nc.sync.dma_start(out=wsb[:, _WMID0:], in_=wpack[:, _WMID0:])

        # views into the packed image
        wm_sb = wsb[:, _WM0:_WC0].rearrange("p (k j) -> p k j", j=E)
        wc_sb = wsb[:, _WC0:_WMID0].rearrange("p (k j) -> p k j", j=E)
        wmid_sb = wsb[:, _WMID0:_C2W0].rearrange("p (k j) -> p k j", j=E)
        c2w_sb = wsb[:, _C2W0:_C3W0].rearrange("p (k j) -> p k j", j=64)
        c3w_sb = wsb[:64, _C3W0:_C3W0 + 1]
        d_sb = wsb[:1, _D0:_D0 + E]
        ones_sb = wsb[:1, _ONES0:_ONES0 + 128]
        ident = wsb[:, _ID0:_ID0 + 128]
        g_sb = [fsb[:, 0:1], fsb[:, 1:2]]
        be_sb = [fsb[:, 2:3], fsb[:, 3:4]]
        eps_sb = fsb[:, 4:5]
        c2b_sb = fsb[:64, 5:6]
        c3b_sb = fsb[:1, 6:7]

        def ln_rs(ph, sbuf_src=False):
            """rsqrt(var+eps) of a [128, E] fp32 tile, per partition."""
            stats = spool.tile([128, 6], _F32, tag="stats")
            nc.vector.bn_stats(out=stats, in_=ph)
            mv = spool.tile([128, 2], _F32, tag="mv")
            nc.vector.bn_aggr(out=mv, in_=stats)
            sd = spool.tile([128, 1], _F32, tag="sd")
            nc.scalar.activation(
                out=sd, in_=mv[:, 1:2],
                func=mybir.ActivationFunctionType.Sqrt,
                bias=eps_sb, scale=1.0,
            )
            rs = spool.tile([128, 1], _F32, tag="rs")
            nc.vector.reciprocal(out=rs, in_=sd)
            return rs

        d_bc = fsb[:, 8:8 + E]
        out_tiles = {}   # st -> out_sb tile

        def stage_a(st, bb, x_m, x_c):
            """Embed matmuls + LN chain + z evict."""
            bcol = bass.ts(bb, 128)
            pmc = pe_mc.tile([128, 2, E], _F32, tag="mc")
            zs = []
            for i, (x_sb, w_sb) in enumerate(((x_m, wm_sb), (x_c, wc_sb))):
                for k in range(KCH):
                    nc.tensor.matmul(
                        pmc[:, i, :], lhsT=x_sb[:, k, bcol], rhs=w_sb[:, k, :],
                        start=(k == 0), stop=(k == KCH - 1))
            for i in range(2):
                rs = ln_rs(pmc[:, i, :])
                z = zpool.tile([128, E], _BF, tag=f"z{i}")
                nc.scalar.activation(
                    out=z, in_=pmc[:, i, :],
                    func=mybir.ActivationFunctionType.Relu,
                    scale=rs)
                zs.append(z)
            return {"st": st, "bb": bb, "z": zs}

        def stage_b(ctxb):
            """z transposes on PE + copybacks -> hT."""
            pt = pe_t.tile([128, 4, 128], _BF, tag="pt")
            hts = []
            for i in range(2):
                z = ctxb["z"][i]
                for chv in range(2):
                    nc.tensor.transpose(
                        pt[:, 2 * i + chv, :], z[:, bass.ts(chv, 128)], ident)
                ht = hpool.tile([128, 2, 128], _BF, tag=f"hT{i}")
                hts.append(ht)
            for i in range(2):
                nc.scalar.copy(out=hts[i][:, 0, :], in_=pt[:, 2 * i, :])
                nc.vector.tensor_copy(out=hts[i][:, 1, :], in_=pt[:, 2 * i + 1, :])
            ctxb["ht"] = hts

        def stage_c(ctxb):
            """Mid matmuls + d add + z1 chain."""
            ht_m, ht_c = ctxb["ht"]
            p1 = pe_p1.tile([128, E], _F32, tag="p1")
            nc.tensor.matmul(p1, lhsT=ht_c[:, 0, :], rhs=wmid_sb[:, 0, :], start=True, stop=False)
            nc.tensor.matmul(p1, lhsT=ht_c[:, 1, :], rhs=wmid_sb[:, 1, :], start=False, stop=False)
            nc.tensor.matmul(p1, lhsT=ht_m[:, 0, :], rhs=wmid_sb[:, 2, :], start=False, stop=False)
            nc.tensor.matmul(p1, lhsT=ht_m[:, 1, :], rhs=wmid_sb[:, 3, :], start=False, stop=True)
            s1 = spool.tile([128, E], _F32, tag="s1")
            nc.vector.tensor_add(out=s1, in0=p1, in1=d_bc)
            rs1 = ln_rs(s1)
            z1 = zpool.tile([128, E], _BF, tag="zmid")
            nc.vector.tensor_scalar_mul(out=z1, in0=s1, scalar1=rs1)
            ctxb["z1"] = z1

        def stage_d(ctxb):
            """z1 transposes on PE + h1 = relu(g*z1T + be) copyback."""
            z1 = ctxb["z1"]
            pt = pe_t.tile([128, 4, 128], _BF, tag="pt")
            h1 = hpool.tile([128, 2, 128], _BF, tag="h1T")
            for chv in range(2):
                nc.tensor.transpose(pt[:, chv, :], z1[:, bass.ts(chv, 128)], ident)
            for chv in range(2):
                nc.scalar.activation(
                    out=h1[:, chv, :], in_=pt[:, chv, :],
                    func=mybir.ActivationFunctionType.Relu,
                    bias=be_sb[chv], scale=g_sb[chv])
            ctxb["h1"] = h1

        h2cats = {}  # st -> [64, ST] bf16 accumulator

        def stage_e(ctxb):
            """c2 matmul + h2 evict into the per-supertile batch tile."""
            st, bb = ctxb["st"], ctxb["bb"]
            h1 = ctxb["h1"]
            ph2 = pe_small.tile([64, 128], _F32, tag="small")
            nc.tensor.matmul(ph2, lhsT=c2w_sb[:, 0, :], rhs=h1[:, 0, :], start=True, stop=False)
            nc.tensor.matmul(ph2, lhsT=c2w_sb[:, 1, :], rhs=h1[:, 1, :], start=False, stop=True)
            if bb == 0:
                h2cats[st] = hpool.tile([64, ST], _BF, tag="h2T", name="h2cat")
            nc.scalar.activation(
                out=h2cats[st][:, bass.ts(bb, 128)], in_=ph2,
                func=mybir.ActivationFunctionType.Relu,
                bias=c2b_sb)

        def stage_f(ctxb):
            """Batched c3 matmul over a whole supertile + output write."""
            st, bb = ctxb["st"], ctxb["bb"]
            if bb != NBB - 1:
                return
            po = pe_small.tile([1, ST], _F32, tag="small")
            nc.tensor.matmul(po, lhsT=c3w_sb, rhs=h2cats.pop(st), start=True, stop=True)
            nc.vector.tensor_scalar_add(
                out=out_tiles[st], in0=po, scalar1=c3b_sb)
            nc.sync.dma_start(
                out=out[:, bass.ts(st, ST)], in_=out_tiles.pop(st))

        stages = [stage_b, stage_c, stage_d, stage_e, stage_f]
        pipe = []
        for st in range(NST):
            cols = bass.ts(st, ST)
            x_m = xpool.tile([128, KCH, ST], _BF, tag="x_m")
            x_c = xpool.tile([128, KCH, ST], _BF, tag="x_c")
            if st == 0:
                # block 0 first, then weights, then the remaining blocks —
                # so the first embed matmuls start after ~1 MB, not ~7 MB.
                for bb in range(NBB):
                    bcol = bass.ts(bb, 128)
                    for xt, xd in ((x_m, xm), (x_c, xc)):
                        nc.sync.dma_start(
                            out=xt[:, :, bcol],
                            in_=xd[:, cols][:, bcol].rearrange(
                                "(k p) c -> p k c", p=128))
                    if bb == 0:
                        load_consts()
            else:
                nc.sync.dma_start(
                    out=x_m, in_=xm[:, cols].rearrange("(k p) c -> p k c", p=128))
                nc.sync.dma_start(
                    out=x_c, in_=xc[:, cols].rearrange("(k p) c -> p k c", p=128))
            out_tiles[st] = opool.tile([1, ST], _F32, tag="out_sb", name="out_sb")

            for bb in range(NBB):
                pipe.append(stage_a(st, bb, x_m, x_c))
                for depth, fn in enumerate(stages, start=2):
                    if len(pipe) >= depth:
                        fn(pipe[-depth])
                if len(pipe) > len(stages):
                    pipe.pop(0)
        # drain pipeline: stage k (b..f) still owes the last k+1 blocks
        for k, fn in enumerate(stages):
            for ctxb in pipe[-(k + 1):]:
                fn(ctxb)

    return nc


def _legalize_waits(nc):
    """Split multi-semaphore waits: this walrus build accepts at most one
    sync-wait per instruction (two on EventSemaphore), so excess waits are
    hoisted into preceding EventSemaphore instructions on the same engine."""
    for bb in nc.main_func.blocks:
        new_insts = []
        changed = False
        for inst in bb.instructions:
            si = inst.sync_info
            if si is not None and si.on_wait:
                cap = 2 if isinstance(inst, mybir.InstEventSemaphore) else 1
                waits = list(si.on_wait)
                while len(waits) > cap:
                    spill, waits = waits[:2], waits[2:]
                    ev = mybir.InstEventSemaphore(
                        name=nc.get_next_instruction_name(),
                        ins=[], outs=[],
                        engine=inst.engine,
                        sync_info=mybir.SyncInfo(on_wait=spill, on_update=[]),
                    )
                    new_insts.append(ev)
                    changed = True
                si.on_wait = waits
            new_insts.append(inst)
        if changed:
            bb.instructions[:] = new_insts


_NC_CACHE = {}


def _get_nc():
    if "nc" not in _NC_CACHE:
        nc = _build_nc()
        _legalize_waits(nc)
        _NC_CACHE["nc"] = nc
    return _NC_CACHE["nc"]


def _fold_weights(inp):
    f8 = lambda k: np.asarray(inp[k]).astype(np.float64)
    P_c = np.eye(E) - 1.0 / E

    me_w, me_b = f8("me_w"), f8("me_b")
    ce_w, ce_b = f8("ce_w"), f8("ce_b")
    Wm = np.zeros((KP, E))
    Wm[:IN_DIM] = me_w @ P_c
    Wm[IN_DIM] = me_b @ P_c
    Wc = np.zeros((KP, E))
    Wc[:IN_DIM] = ce_w @ P_c
    Wc[IN_DIM] = ce_b @ P_c

    c1_w, c1_b = f8("c1_w"), f8("c1_b")
    A0 = f8("ca_in_w")[:, 2 * E:] @ f8("ca_out_w")
    a0 = f8("ca_in_b")[2 * E:] @ f8("ca_out_w") + f8("ca_out_b")
    S0 = f8("sa_in_w")[:, 2 * E:] @ f8("sa_out_w")
    s0 = f8("sa_in_b")[2 * E:] @ f8("sa_out_w") + f8("sa_out_b")
    CA = (A0 @ c1_w[:E]) @ P_c
    CM = (S0 @ c1_w[E:]) @ P_c
    d = (a0 @ c1_w[:E] + s0 @ c1_w[E:] + c1_b) @ P_c

    # ---- bf16 packed image ----
    w = np.zeros((128, _WCOLS), BF16)

    def chunked(mat, ncols):       # [k*128, ncols] -> [128, k*ncols]
        k = mat.shape[0] // 128
        return mat.reshape(k, 128, ncols).transpose(1, 0, 2).reshape(128, k * ncols)

    w[:, _WM0:_WC0] = chunked(Wm, E).astype(BF16)
    w[:, _WC0:_WMID0] = chunked(Wc, E).astype(BF16)
    w[:, _WMID0:_C2W0] = chunked(np.vstack([CA, CM]), E).astype(BF16)
    w[:, _C2W0:_C3W0] = chunked(f8("c2_w"), 64).astype(BF16)
    w[:64, _C3W0:_C3W0 + 1] = f8("c3_w").astype(BF16)
    w[0, _D0:_D0 + E] = d.astype(BF16)
    w[0, _ONES0:_ONES0 + 128] = 1
    w[:, _ID0:_ID0 + 128] = np.eye(128, dtype=BF16)

    # ---- f32 packed image ----
    f = np.zeros((128, _FCOLS), F32)
    g = np.asarray(inp["c1_g"]).astype(F32)
    be = np.asarray(inp["c1_be"]).astype(F32)
    f[:, 0] = g[:128]
    f[:, 1] = g[128:]
    f[:, 2] = be[:128]
    f[:, 3] = be[128:]
    f[:, 4] = EPS
    f[:64, 5] = np.asarray(inp["c2_b"]).astype(F32)
    f[0, 6] = float(np.asarray(inp["c3_b"]).reshape(-1)[0])
    f[:, 8:8 + E] = d.astype(F32)[None, :]
    return {"wpack": w, "fpack": f}


def _shard_x(x):
    """x [B, 2, IN_DIM] f32 -> per-core transposed bf16 [KP, ROWS] pairs."""
    maps = []
    for i in range(N_CORES):
        sl = x[i * ROWS:(i + 1) * ROWS]          # [ROWS, 2, IN_DIM]
        xm = np.zeros((KP, ROWS), BF16)
        xc = np.zeros((KP, ROWS), BF16)
        xm[:IN_DIM] = np.ascontiguousarray(sl[:, 0, :]).astype(BF16).T
        xm[IN_DIM] = 1
        xc[:IN_DIM] = np.ascontiguousarray(sl[:, 1, :]).astype(BF16).T
        xc[IN_DIM] = 1
        maps.append((xm, xc))
    return maps


def kernel(**inputs):
    x = np.asarray(inputs["x"], dtype=np.float32)
    weights = _fold_weights(inputs)
    shards = _shard_x(x)
    in_maps = [{"xm": xm, "xc": xc, **weights} for xm, xc in shards]

    nc = _get_nc()
    res = run_bass_kernel_spmd(nc, in_maps, list(range(N_CORES)))
    outs = [np.asarray(r["out"]).reshape(ROWS) for r in res.results]
    return np.concatenate(outs).reshape(B, 1).astype(np.float32)


# revision 18
# speedup vs baseline: 1.1343x; 1.1343x over previous
"""Trainium2 Bass kernel for nn_CrossAttentionClassifier.

Strategy
--------
The reference network with q_len = kv_len = 1 attention degenerates into a
pure MLP:

    z_m = mut @ Wm' + bm'          (centered: LN mean-subtraction folded into W)
    z_c = ctx @ Wc' + bc'
    h_m = relu(z_m * rsqrt(mean(z_m^2)+eps)) ; h_c likewise
    pre1 = h_c @ CA + h_m @ CM + d (attention V/out projections + c1 folded)
    z1   = pre1 * rsqrt(mean(pre1^2)+eps)
    h1   = relu(z1 * g + be)
    h2   = relu(h1 @ c2_w + c2_b)
    out  = h2 @ c3_w + c3_b

All weight folding (products of the tiny 256x256 projection chains and the
centering projector I - 11^T/256) happens on host in float64; the batch-heavy
work runs on 8 NeuronCores, data-parallel over the 65536-row batch.

Device layout: batch on SBUF partitions, features on the free axis, so both
layernorms reduce along the free dimension (bn_stats).  Activations are
transposed 128x128 via the PE between layers so the next matmul's contraction
dim lands on partitions.  Biases of the big embed matmuls ride along as an
appended constant-one feature row.  All matmuls run in bf16 (fp32 PSUM
accumulate).

The per-block work is software-pipelined three deep (A: embed matmuls,
B: z transposes + mid matmul, C: z1 transposes + c2/c3) so the Tensor
engine's in-order stream never waits on the DVE/ACT layernorm chains —
keeping PE HAM-warm.  All constants arrive in two packed DMAs (walrus
allows only one sync-wait per instruction; see _legalize_waits).
"""

import numpy as np
import ml_dtypes

import concourse.bass as bass
import concourse.mybir as mybir
import concourse.tile as tile
from concourse.bass_utils import run_bass_kernel_spmd

BF16 = ml_dtypes.bfloat16
F32 = np.float32

N_CORES = 8
B = 65536
IN_DIM = 2056
E = 256
EPS = 1e-5
KP = 2176          # feature dim padded to 17*128 (incl. bias row at 2056)
KCH = KP // 128    # 17
ROWS = B // N_CORES   # 8192 rows per core
ST = 512           # batch columns per supertile
NST = ROWS // ST   # 16
NBB = ST // 128    # 4 blocks of 128 per supertile

_BF = mybir.dt.bfloat16
_F32 = mybir.dt.float32

# ---- packed bf16 const image column offsets ----
_WM0 = 0
_WC0 = _WM0 + KCH * E          # 4352
_WMID0 = _WC0 + KCH * E        # 8704
_C2W0 = _WMID0 + 4 * E         # 9728
_C3W0 = _C2W0 + 2 * 64         # 9856
_D0 = _C3W0 + 1                # 9857
_ONES0 = _D0 + E               # 10113
_ID0 = _ONES0 + 128            # 10241
_WCOLS = _ID0 + 128            # 10369

# ---- packed f32 const image columns ----
# 0: g chunk0, 1: g chunk1, 2: be chunk0, 3: be chunk1, 4: eps,
# 5: c2_b (64 valid), 6: c3_b (1 valid), 7: pad, 8:8+E: d broadcast
_FCOLS = 8 + E


def _build_nc():
    nc = bass.Bass()

    xm = nc.dram_tensor("xm", [KP, ROWS], _BF, kind="ExternalInput")
    xc = nc.dram_tensor("xc", [KP, ROWS], _BF, kind="ExternalInput")
    wpack = nc.dram_tensor("wpack", [128, _WCOLS], _BF, kind="ExternalInput")
    fpack = nc.dram_tensor("fpack", [128, _FCOLS], _F32, kind="ExternalInput")
    out = nc.dram_tensor("out", [1, ROWS], _F32, kind="ExternalOutput")

    from contextlib import ExitStack

    with tile.TileContext(nc) as tc, ExitStack() as ctx:
        consts = ctx.enter_context(tc.tile_pool(name="consts", bufs=1))
        xpool = ctx.enter_context(tc.tile_pool(name="xpool", bufs=2))
        zpool = ctx.enter_context(tc.tile_pool(name="zpool", bufs=3))
        hpool = ctx.enter_context(tc.tile_pool(name="hpool", bufs=4))
        spool = ctx.enter_context(tc.tile_pool(name="spool", bufs=6))
        opool = ctx.enter_context(tc.tile_pool(name="opool", bufs=3))
        pe_mc = ctx.enter_context(tc.tile_pool(name="pe_mc", bufs=2, space="PSUM"))
        pe_p1 = ctx.enter_context(tc.tile_pool(name="pe_p1", bufs=2, space="PSUM"))
        pe_t = ctx.enter_context(tc.tile_pool(name="pe_t", bufs=2, space="PSUM"))
        pe_small = ctx.enter_context(tc.tile_pool(name="pe_small", bufs=2, space="PSUM"))

        wsb = consts.tile([128, _WCOLS], _BF, tag="wsb")
        fsb = consts.tile([128, _FCOLS], _F32, tag="fsb")

        def load_consts():
            # wm/wc in quarters so the first embed matmuls start early;
            # emitted interleaved with supertile 0's per-block x loads.
            qs = [(0, 5), (5, 9), (9, 13), (13, KCH)]
            for base in (_WM0, _WC0):
                for k0, k1 in qs:
                    nc.sync.dma_start(
                        out=wsb[:, base + k0 * E:base + k1 * E],
                        in_=wpack[:, base + k0 * E:base + k1 * E])
            nc.sync.dma_start(out=fsb, in_=fpack[:])
            nc.sync.dma_start(out=wsb[:, _WMID0:], in_=wpack[:, _WMID0:])

        # views into the packed image
        wm_sb = wsb[:, _WM0:_WC0].rearrange("p (k j) -> p k j", j=E)
        wc_sb = wsb[:, _WC0:_WMID0].rearrange("p (k j) -> p k j", j=E)
        wmid_sb = wsb[:, _WMID0:_C2W0].rearrange("p (k j) -> p k j", j=E)
        c2w_sb = wsb[:, _C2W0:_C3W0].rearrange("p (k j) -> p k j", j=64)
        c3w_sb = wsb[:64, _C3W0:_C3W0 + 1]
        d_sb = wsb[:1, _D0:_D0 + E]
        ones_sb = wsb[:1, _ONES0:_ONES0 + 128]
        ident = wsb[:, _ID0:_ID0 + 128]
        g_sb = [fsb[:, 0:1], fsb[:, 1:2]]
        be_sb = [fsb[:, 2:3], fsb[:, 3:4]]
        eps_sb = fsb[:, 4:5]
        c2b_sb = fsb[:64, 5:6]
        c3b_sb = fsb[:1, 6:7]

        def ln_rs(ph, sbuf_src=False):
            """rsqrt(var+eps) of a [128, E] fp32 tile, per partition."""
            stats = spool.tile([128, 6], _F32, tag="stats")
            nc.vector.bn_stats(out=stats, in_=ph)
            mv = spool.tile([128, 2], _F32, tag="mv")
            nc.vector.bn_aggr(out=mv, in_=stats)
            sd = spool.tile([128, 1], _F32, tag="sd")
            nc.scalar.activation(
                out=sd, in_=mv[:, 1:2],
                func=mybir.ActivationFunctionType.Sqrt,
                bias=eps_sb, scale=1.0,
            )
            rs = spool.tile([128, 1], _F32, tag="rs")
            nc.vector.reciprocal(out=rs, in_=sd)
            return rs

        d_bc = fsb[:, 8:8 + E]
        out_tiles = {}   # st -> out_sb tile

        def stage_a(st, bb, x_m, x_c):
            """Embed matmuls + LN chain + z evict."""
            bcol = bass.ts(bb, 128)
            pmc = pe_mc.tile([128, 2, E], _F32, tag="mc")
            zs = []
            for i, (x_sb, w_sb) in enumerate(((x_m, wm_sb), (x_c, wc_sb))):
                for k in range(KCH):
                    nc.tensor.matmul(
                        pmc[:, i, :], lhsT=x_sb[:, k, bcol], rhs=w_sb[:, k, :],
                        start=(k == 0), stop=(k == KCH - 1))
            for i in range(2):
                rs = ln_rs(pmc[:, i, :])
                z = zpool.tile([128, E], _BF, tag=f"z{i}")
                nc.scalar.activation(
                    out=z, in_=pmc[:, i, :],
                    func=mybir.ActivationFunctionType.Relu,
                    scale=rs)
                zs.append(z)
            return {"st": st, "bb": bb, "z": zs}

        def stage_b(ctxb):
            """z transposes on PE + copybacks -> hT."""
            pt = pe_t.tile([128, 4, 128], _BF, tag="pt")
            hts = []
            for i in range(2):
                z = ctxb["z"][i]
                for chv in range(2):
                    nc.tensor.transpose(
                        pt[:, 2 * i + chv, :], z[:, bass.ts(chv, 128)], ident)
                ht = hpool.tile([128, 2, 128], _BF, tag=f"hT{i}")
                hts.append(ht)
            for i in range(2):
                nc.scalar.copy(out=hts[i][:, 0, :], in_=pt[:, 2 * i, :])
                nc.vector.tensor_copy(out=hts[i][:, 1, :], in_=pt[:, 2 * i + 1, :])
            ctxb["ht"] = hts

        def stage_c(ctxb):
            """Mid matmuls + d add + z1 chain."""
            ht_m, ht_c = ctxb["ht"]
            p1 = pe_p1.tile([128, E], _F32, tag="p1")
            nc.tensor.matmul(p1, lhsT=ht_c[:, 0, :], rhs=wmid_sb[:, 0, :], start=True, stop=False)
            nc.tensor.matmul(p1, lhsT=ht_c[:, 1, :], rhs=wmid_sb[:, 1, :], start=False, stop=False)
            nc.tensor.matmul(p1, lhsT=ht_m[:, 0, :], rhs=wmid_sb[:, 2, :], start=False, stop=False)
            nc.tensor.matmul(p1, lhsT=ht_m[:, 1, :], rhs=wmid_sb[:, 3, :], start=False, stop=True)
            s1 = spool.tile([128, E], _F32, tag="s1")
            nc.vector.tensor_add(out=s1, in0=p1, in1=d_bc)
            rs1 = ln_rs(s1)
            z1 = zpool.tile([128, E], _BF, tag="zmid")
            nc.vector.tensor_scalar_mul(out=z1, in0=s1, scalar1=rs1)
            ctxb["z1"] = z1

        def stage_d(ctxb):
            """z1 transposes on PE + h1 = relu(g*z1T + be) copyback."""
            z1 = ctxb["z1"]
            pt = pe_t.tile([128, 4, 128], _BF, tag="pt")
            h1 = hpool.tile([128, 2, 128], _BF, tag="h1T")
            for chv in range(2):
                nc.tensor.transpose(pt[:, chv, :], z1[:, bass.ts(chv, 128)], ident)
            for chv in range(2):
                nc.scalar.activation(
                    out=h1[:, chv, :], in_=pt[:, chv, :],
                    func=mybir.ActivationFunctionType.Relu,
                    bias=be_sb[chv], scale=g_sb[chv])
            ctxb["h1"] = h1

        h2cats = {}  # st -> [64, ST] bf16 accumulator

        def stage_e(ctxb):
            """c2 matmul + h2 evict into the per-supertile batch tile."""
            st, bb = ctxb["st"], ctxb["bb"]
            h1 = ctxb["h1"]
            ph2 = pe_small.tile([64, 128], _F32, tag="small")
            nc.tensor.matmul(ph2, lhsT=c2w_sb[:, 0, :], rhs=h1[:, 0, :], start=True, stop=False)
            nc.tensor.matmul(ph2, lhsT=c2w_sb[:, 1, :], rhs=h1[:, 1, :], start=False, stop=True)
            if bb == 0:
                h2cats[st] = hpool.tile([64, ST], _BF, tag="h2T", name="h2cat")
            nc.scalar.activation(
                out=h2cats[st][:, bass.ts(bb, 128)], in_=ph2,
                func=mybir.ActivationFunctionType.Relu,
                bias=c2b_sb)

        def stage_f(ctxb):
            """Batched c3 matmul over a whole supertile + output write."""
            st, bb = ctxb["st"], ctxb["bb"]
            if bb != NBB - 1:
                return
            po = pe_small.tile([1, ST], _F32, tag="small")
            nc.tensor.matmul(po, lhsT=c3w_sb, rhs=h2cats.pop(st), start=True, stop=True)
            nc.vector.tensor_scalar_add(
                out=out_tiles[st], in0=po, scalar1=c3b_sb)
            nc.sync.dma_start(
                out=out[:, bass.ts(st, ST)], in_=out_tiles.pop(st))

        stages = [stage_b, stage_c, stage_d, stage_e, stage_f]
        pipe = []
        for st in range(NST):
            cols = bass.ts(st, ST)
            x_m = xpool.tile([128, KCH, ST], _BF, tag="x_m")
            x_c = xpool.tile([128, KCH, ST], _BF, tag="x_c")
            if st == 0:
                # K-chunk pieces interleaved with the weight quarters so the
                # first embed matmuls start after ~1 MB of DMA, not ~7 MB.
                # Chunk-range splits keep the 1KB-per-partition DMA lines.
                pieces = [(0, 4), (4, 9), (9, 13), (13, KCH)]
                for pi, (k0, k1) in enumerate(pieces):
                    for xt, xd in ((x_m, xm), (x_c, xc)):
                        nc.sync.dma_start(
                            out=xt[:, k0:k1, :],
                            in_=xd[k0 * 128:k1 * 128, cols].rearrange(
                                "(k p) c -> p k c", p=128))
                    if pi == 0:
                        load_consts()
            else:
                nc.sync.dma_start(
                    out=x_m, in_=xm[:, cols].rearrange("(k p) c -> p k c", p=128))
                nc.sync.dma_start(
                    out=x_c, in_=xc[:, cols].rearrange("(k p) c -> p k c", p=128))
            out_tiles[st] = opool.tile([1, ST], _F32, tag="out_sb", name="out_sb")

            for bb in range(NBB):
                pipe.append(stage_a(st, bb, x_m, x_c))
                for depth, fn in enumerate(stages, start=2):
                    if len(pipe) >= depth:
                        fn(pipe[-depth])
                if len(pipe) > len(stages):
                    pipe.pop(0)
        # drain pipeline: stage k (b..f) still owes the last k+1 blocks
        for k, fn in enumerate(stages):
            for ctxb in pipe[-(k + 1):]:
                fn(ctxb)

    return nc


def _legalize_waits(nc):
    """Split multi-semaphore waits: this walrus build accepts at most one
    sync-wait per instruction (two on EventSemaphore), so excess waits are
    hoisted into preceding EventSemaphore instructions on the same engine."""
    for bb in nc.main_func.blocks:
        new_insts = []
        changed = False
        for inst in bb.instructions:
            si = inst.sync_info
            if si is not None and si.on_wait:
                cap = 2 if isinstance(inst, mybir.InstEventSemaphore) else 1
                waits = list(si.on_wait)
                while len(waits) > cap:
                    spill, waits = waits[:2], waits[2:]
                    ev = mybir.InstEventSemaphore(
                        name=nc.get_next_instruction_name(),
                        ins=[], outs=[],
                        engine=inst.engine,
                        sync_info=mybir.SyncInfo(on_wait=spill, on_update=[]),
                    )
                    new_insts.append(ev)
                    changed = True
                si.on_wait = waits
            new_insts.append(inst)
        if changed:
            bb.instructions[:] = new_insts


_NC_CACHE = {}


def _get_nc():
    if "nc" not in _NC_CACHE:
        nc = _build_nc()
        _legalize_waits(nc)
        _NC_CACHE["nc"] = nc
    return _NC_CACHE["nc"]


def _fold_weights(inp):
    f8 = lambda k: np.asarray(inp[k]).astype(np.float64)
    P_c = np.eye(E) - 1.0 / E

    me_w, me_b = f8("me_w"), f8("me_b")
    ce_w, ce_b = f8("ce_w"), f8("ce_b")
    Wm = np.zeros((KP, E))
    Wm[:IN_DIM] = me_w @ P_c
    Wm[IN_DIM] = me_b @ P_c
    Wc = np.zeros((KP, E))
    Wc[:IN_DIM] = ce_w @ P_c
    Wc[IN_DIM] = ce_b @ P_c

    c1_w, c1_b = f8("c1_w"), f8("c1_b")
    A0 = f8("ca_in_w")[:, 2 * E:] @ f8("ca_out_w")
    a0 = f8("ca_in_b")[2 * E:] @ f8("ca_out_w") + f8("ca_out_b")
    S0 = f8("sa_in_w")[:, 2 * E:] @ f8("sa_out_w")
    s0 = f8("sa_in_b")[2 * E:] @ f8("sa_out_w") + f8("sa_out_b")
    CA = (A0 @ c1_w[:E]) @ P_c
    CM = (S0 @ c1_w[E:]) @ P_c
    d = (a0 @ c1_w[:E] + s0 @ c1_w[E:] + c1_b) @ P_c

    # ---- bf16 packed image ----
    w = np.zeros((128, _WCOLS), BF16)

    def chunked(mat, ncols):       # [k*128, ncols] -> [128, k*ncols]
        k = mat.shape[0] // 128
        return mat.reshape(k, 128, ncols).transpose(1, 0, 2).reshape(128, k * ncols)

    w[:, _WM0:_WC0] = chunked(Wm, E).astype(BF16)
    w[:, _WC0:_WMID0] = chunked(Wc, E).astype(BF16)
    w[:, _WMID0:_C2W0] = chunked(np.vstack([CA, CM]), E).astype(BF16)
    w[:, _C2W0:_C3W0] = chunked(f8("c2_w"), 64).astype(BF16)
    w[:64, _C3W0:_C3W0 + 1] = f8("c3_w").astype(BF16)
    w[0, _D0:_D0 + E] = d.astype(BF16)
    w[0, _ONES0:_ONES0 + 128] = 1
    w[:, _ID0:_ID0 + 128] = np.eye(128, dtype=BF16)

    # ---- f32 packed image ----
    f = np.zeros((128, _FCOLS), F32)
    g = np.asarray(inp["c1_g"]).astype(F32)
    be = np.asarray(inp["c1_be"]).astype(F32)
    f[:, 0] = g[:128]
    f[:, 1] = g[128:]
    f[:, 2] = be[:128]
    f[:, 3] = be[128:]
    f[:, 4] = EPS
    f[:64, 5] = np.asarray(inp["c2_b"]).astype(F32)
    f[0, 6] = float(np.asarray(inp["c3_b"]).reshape(-1)[0])
    f[:, 8:8 + E] = d.astype(F32)[None, :]
    return {"wpack": w, "fpack": f}


def _shard_x(x):
    """x [B, 2, IN_DIM] f32 -> per-core transposed bf16 [KP, ROWS] pairs."""
    maps = []
    for i in range(N_CORES):
        sl = x[i * ROWS:(i + 1) * ROWS]          # [ROWS, 2, IN_DIM]
        xm = np.zeros((KP, ROWS), BF16)
        xc = np.zeros((KP, ROWS), BF16)
        xm[:IN_DIM] = np.ascontiguousarray(sl[:, 0, :]).astype(BF16).T
        xm[IN_DIM] = 1
        xc[:IN_DIM] = np.ascontiguousarray(sl[:, 1, :]).astype(BF16).T
        xc[IN_DIM] = 1
        maps.append((xm, xc))
    return maps


def kernel(**inputs):
    x = np.asarray(inputs["x"], dtype=np.float32)
    weights = _fold_weights(inputs)
    shards = _shard_x(x)
    in_maps = [{"xm": xm, "xc": xc, **weights} for xm, xc in shards]

    nc = _get_nc()
    res = run_bass_kernel_spmd(nc, in_maps, list(range(N_CORES)))
    outs = [np.asarray(r["out"]).reshape(ROWS) for r in res.results]
    return np.concatenate(outs).reshape(B, 1).astype(np.float32)
